# revision 1
# baseline (speedup 1.0000x reference)
"""Trainium2 Bass kernel: grouped-experts SwiGLU MLP with mid-RMSNorm.

Expert-parallel across 8 NeuronCores: core e computes expert e's token
block (tokens are pre-sorted by expert).  Host gathers each expert's
rows into a zero-padded [C, D] buffer, ships transposed activations and
weights, and scatters the per-core outputs back to flat token order.

Per-core math (all fp16 in / fp32 accumulate):
    h1 = x @ w1^T ; h3 = x @ w3^T          # [C, F]
    h  = silu(h1) * h3
    h  = h * rsqrt(mean(h^2) + eps)        # RMSNorm (scale folded to out)
    out = (h * mid_w) @ w2^T               # mid_w folded into w2 on host

DMA notes: per-queue bandwidth is a fraction of the ~358 GB/s HBM
aggregate, so the streams are spread over the three DMA-capable queues
(sync/gpsimd/scalar) with the first f-block's weights issued in
consumption order, greedily balanced.  All host-side tensors are
pre-arranged so every DMA slab is >=2KB-contiguous per partition.
"""

import sys

sys.path.insert(0, "/opt/trn_rl_repo")

import numpy as np
from contextlib import ExitStack

import os

import concourse.bass as bass
import concourse.tile as tile
from concourse import bacc, mybir
from concourse.masks import make_identity

P = 128
T = 4096
D = 2048
F = 1024
E = 8
NB = 512  # matmul moving-dim block (one PSUM bank of fp32)
EPS = 1e-6
F32 = mybir.dt.float32
F16 = mybir.dt.float16
ACTF = mybir.ActivationFunctionType

_PROGRAM_CACHE: dict[int, object] = {}
LAST_RESULTS = None  # test harness reads per-core outputs from here


def _run(nc, in_maps):
    """Execute the compiled program on the 8 axon-tunneled cores.

    If KERNEL_NTFF_DIR is set, wrap the execute in the axon NTFF profile
    hook so device profiles land there (test harness use only).
    """
    from concourse import bass2jax

    ntff_dir = os.environ.get("KERNEL_NTFF_DIR")
    if ntff_dir:
        if "/root/.axon_site" not in sys.path:
            sys.path.insert(0, "/root/.axon_site")
        from trn_agent_boot.trn_boot import _ntff_profile_via_ctypes

        hook = _ntff_profile_via_ctypes("/opt/axon/libaxon_pjrt.so")
        ids = [
            int(x) for x in os.environ.get("KERNEL_NTFF_CORES", "0").split(",")
        ]
        if hook is not None:
            with hook(ntff_dir, ids):
                return bass2jax.run_bass_via_pjrt(nc, in_maps, n_cores=len(in_maps))
    return bass2jax.run_bass_via_pjrt(nc, in_maps, n_cores=len(in_maps))


def _tile_stats_and_transpose(nc, qpool, ps_t, h_tiles, ht_tiles, ssq_all, ident, t):
    KF = len(ht_tiles)
    hsq = qpool.tile([P, h_tiles[t].shape[1]], F32, tag="hsq", name=f"hsq{t}")
    nc.scalar.activation(
        hsq[:], h_tiles[t][:], ACTF.Square, accum_out=ssq_all[:, t : t + 1]
    )
    for fc in range(KF):
        pst = ps_t.tile([P, P], F16, tag="tp", name=f"pst{t}_{fc}")
        nc.tensor.transpose(pst[:], h_tiles[t][:, fc * P : (fc + 1) * P], ident[:])
        nc.vector.tensor_copy(ht_tiles[fc][:, t * P : (t + 1) * P], pst[:])


def _build_program(C: int):
    """Build + compile the single-core SPMD program for C padded rows."""
    NT = C // P  # token tiles per core
    KD = D // P  # 16 contraction chunks for mm1
    KF = F // P  # 8 contraction chunks for mm2
    FB = F // NB  # 2 f-blocks
    DB = D // NB  # 4 d-blocks

    nc = bacc.Bacc(
        "TRN2",
        target_bir_lowering=False,
        debug=False,
        enable_asserts=False,
        num_devices=E,
    )
    xT_d = nc.dram_tensor("xT", [P, NT, KD, P], F16, kind="ExternalInput").ap()
    w1_d = nc.dram_tensor("w1t", [P, FB, KD, NB], F16, kind="ExternalInput").ap()
    w3_d = nc.dram_tensor("w3t", [P, FB, KD, NB], F16, kind="ExternalInput").ap()
    w2_d = nc.dram_tensor("w2t", [P, DB, KF, NB], F16, kind="ExternalInput").ap()
    out_d = nc.dram_tensor("out", [C, D], F16, kind="ExternalOutput").ap()

    with tile.TileContext(nc) as tc, ExitStack() as ctx:
        singles = ctx.enter_context(tc.tile_pool(name="singles", bufs=1))
        xpool = ctx.enter_context(tc.tile_pool(name="x", bufs=1))
        w1pool = ctx.enter_context(tc.tile_pool(name="w1", bufs=2))
        w3pool = ctx.enter_context(tc.tile_pool(name="w3", bufs=2))
        w2pool = ctx.enter_context(tc.tile_pool(name="w2", bufs=4))
        hpool = ctx.enter_context(tc.tile_pool(name="h", bufs=1))
        htpool = ctx.enter_context(tc.tile_pool(name="ht", bufs=1))
        spool = ctx.enter_context(tc.tile_pool(name="scr", bufs=2))
        qpool = ctx.enter_context(tc.tile_pool(name="sq", bufs=1))
        opool = ctx.enter_context(tc.tile_pool(name="o", bufs=8))
        stat = ctx.enter_context(tc.tile_pool(name="stat", bufs=1))
        ps_h = ctx.enter_context(tc.tile_pool(name="psh", bufs=2, space="PSUM"))
        ps_t = ctx.enter_context(tc.tile_pool(name="pst", bufs=2, space="PSUM"))
        ps_o = ctx.enter_context(tc.tile_pool(name="pso", bufs=2, space="PSUM"))

        ident = singles.tile([P, P], F16)
        make_identity(nc, ident[:])
        eps_t = singles.tile([P, 1], F32, name="epsT")
        nc.gpsimd.memset(eps_t[:], EPS)
        warm = singles.tile([P, NB], F16, name="warm")
        nc.gpsimd.memset(warm[:], 0.5)

        xt = xpool.tile([P, NT, KD, P], F16)

        # ---- prologue DMA schedule: fb0 weights + x tiles in consumption
        # order, greedily balanced over the three DMA-capable queues
        # (~0.1 TB/s each).  The scalar queue is capped so its phase-A
        # sigmoids are not pushed past the PSUM-release deadline.
        queues = [nc.sync, nc.gpsimd, nc.scalar]
        qload = [0, 0, 0]
        SCALAR_CAP = 1_700_000

        def issue(dst, src, nbytes, qi=None):
            if qi is None:
                elig = [0, 1] + ([2] if qload[2] < SCALAR_CAP else [])
                qi = min(elig, key=lambda i: qload[i])
            queues[qi].dma_start(dst, src)
            qload[qi] += nbytes

        w_tiles = {}
        for fb in range(FB):
            w_tiles[fb] = (
                w1pool.tile([P, KD, NB], F16, tag="w1", name=f"w1h{fb}"),
                w3pool.tile([P, KD, NB], F16, tag="w3", name=f"w3h{fb}"),
            )
        w2_tiles = [
            w2pool.tile([P, KF, NB], F16, tag="w2", name=f"w2b{db}")
            for db in range(DB)
        ]

        wunit2 = 2 * NB * P * 2
        wunit4 = 4 * NB * P * 2
        xunit = (KD // 2) * P * P * 2
        w1h0, w3h0 = w_tiles[0]
        # x tile 0 first on scalar, small leading chunk (gates the very
        # first matmuls), then the rest.
        issue(xt[:, 0, 0:4, :], xT_d[:, 0, 0:4, :], xunit // 2, qi=2)
        issue(xt[:, 0, 4:KD, :], xT_d[:, 0, 4:KD, :], 3 * xunit // 2, qi=2)
        # fb0 weights in consumption order: small k0/k1 units for an early
        # start, then 512KB units (fewer descriptors), greedily balanced.
        # (Every deviation tried — pinning k0-3 to sync, rate-weighted
        # greedy, full hand schedules — measured worse: parallel first
        # units across all three queues beats serial front-loading.)
        for j in range(2):
            ks = bass.ds(j * 2, 2)
            issue(w1h0[:, ks, :], w1_d[:, 0, ks, :], wunit2)
            issue(w3h0[:, ks, :], w3_d[:, 0, ks, :], wunit2)
        for j in range(1, KD // 4):
            ks = bass.ds(j * 4, 4)
            issue(w1h0[:, ks, :], w1_d[:, 0, ks, :], wunit4)
            issue(w3h0[:, ks, :], w3_d[:, 0, ks, :], wunit4)
        # remaining x tiles (whole-tile units)
        for t in range(1, NT):
            issue(xt[:, t], xT_d[:, t], 2 * xunit)
        # fb1 weights ride sync/gpsimd right behind the prologue (the
        # scalar queue must be free for phase-A sigmoids by then).
        for fb in range(1, FB):
            w1h, w3h = w_tiles[fb]
            for j in range(4):
                ks = bass.ds(j * 4, 4)
                issue(w1h[:, ks, :], w1_d[:, fb, ks, :], wunit4, qi=0)
                issue(w3h[:, ks, :], w3_d[:, fb, ks, :], wunit4, qi=1)
        # all w2 up front on sync/gpsimd too — they must not queue behind
        # phase C's output DMAs, and both queues are free by mid-phase-A.
        for db in range(DB):
            w2b = w2_tiles[db]
            issue(w2b[:, 0 : KF // 2, :], w2_d[:, db, 0 : KF // 2, :], wunit4, qi=0)
            issue(w2b[:, KF // 2 :, :], w2_d[:, db, KF // 2 :, :], wunit4, qi=1)

        # HAM warm-up: ~10 dummy matmuls on the scratch tile keep the PE
        # busy through the prologue DMA wait, so the clock gate is already
        # at 8/8 (2.4 GHz) when the first real matmul's data lands.  They
        # need no DMA inputs and their PSUM output is never read.
        ps_w = ps_h.tile([P, NB], F32, tag="ps1", name="warm_ps")
        for _ in range(10):
            nc.tensor.matmul(ps_w[:], warm[:, 0:P], warm[:], start=True, stop=True)

        h_tiles = [hpool.tile([P, F], F16, tag=f"h{t}", name=f"h{t}") for t in range(NT)]
        ht_tiles = [
            htpool.tile([P, C], F16, tag=f"ht{fc}", name=f"ht{fc}")
            for fc in range(KF)
        ]
        ssq_all = stat.tile([P, NT], F32, name="ssq_all")
        std_all = stat.tile([P, NT], F32, name="std_all")
        rstd_all = stat.tile([P, NT], F32, name="rstd_all")

        # ================= phase A: h1/h3 matmuls + swiglu =================
        # fb1 processes the LAST tile first so its phase-B work (fused one
        # iteration behind) lands early; the dangling tile is NT-2, which
        # phase C visits last.
        fb1_order = [NT - 1] + list(range(NT - 1)) if NT > 1 else [0]
        phc_order = fb1_order

        def _tile_done(t):
            # phase B (stats + transpose) then per-tile rstd, so phase C's
            # epilogue never waits on other tiles' statistics.
            _tile_stats_and_transpose(
                nc, qpool, ps_t, h_tiles, ht_tiles, ssq_all, ident, t
            )
            nc.scalar.activation(
                std_all[:, t : t + 1],
                ssq_all[:, t : t + 1],
                ACTF.Sqrt,
                bias=eps_t[:],
                scale=1.0 / F,
            )
            nc.vector.reciprocal(rstd_all[:, t : t + 1], std_all[:, t : t + 1])

        def _swiglu_epilogue(fb, t, ps1, ps3):
            s = spool.tile([P, NB], F32, tag="silu")
            nc.scalar.activation(s[:], ps1[:], ACTF.Sigmoid)
            hs = h_tiles[t][:, fb * NB : (fb + 1) * NB]
            nc.vector.tensor_mul(hs, s[:], ps1[:])
            nc.vector.tensor_mul(hs, hs, ps3[:])

        for fb in range(FB):
            w1h, w3h = w_tiles[fb]
            order = range(NT) if fb < FB - 1 else fb1_order
            for ti, t in enumerate(order):
                ps1 = ps_h.tile([P, NB], F32, tag="ps1")
                ps3 = ps_h.tile([P, NB], F32, tag="ps3")
                for k in range(KD):
                    xs = xt[:, t, k, :]
                    nc.tensor.matmul(
                        ps1[:], xs, w1h[:, k, :], start=(k == 0), stop=(k == KD - 1)
                    )
                    nc.tensor.matmul(
                        ps3[:], xs, w3h[:, k, :], start=(k == 0), stop=(k == KD - 1)
                    )
                _swiglu_epilogue(fb, t, ps1, ps3)
                if fb == FB - 1 and ti >= 1:
                    _tile_done(order[ti - 1])
        _tile_done(fb1_order[-1])

        # ================= phase C: out = hT.T @ w2T, scaled by rstd =======
        for db in range(DB):
            w2b = w2_tiles[db]
            for ti, t in enumerate(phc_order):
                pso = ps_o.tile([P, NB], F32, tag="po")
                for fc in range(KF):
                    nc.tensor.matmul(
                        pso[:],
                        ht_tiles[fc][:, t * P : (t + 1) * P],
                        w2b[:, fc, :],
                        start=(fc == 0),
                        stop=(fc == KF - 1),
                    )
                ob = opool.tile([P, NB], F16, tag="ob")
                nc.vector.tensor_scalar_mul(ob[:], pso[:], rstd_all[:, t : t + 1])
                oq = nc.sync if ti % 2 == 0 else nc.gpsimd
                oq.dma_start(
                    out_d[t * P : (t + 1) * P, db * NB : (db + 1) * NB], ob[:]
                )

    nc.compile()
    return nc


def _get_program(C: int):
    if C not in _PROGRAM_CACHE:
        _PROGRAM_CACHE[C] = _build_program(C)
    return _PROGRAM_CACHE[C]


def kernel(x, w1, w2, w3, mid_w, num_tokens_per_expert):
    global LAST_RESULTS
    x = np.ascontiguousarray(np.asarray(x, dtype=np.float32))
    w1 = np.asarray(w1, dtype=np.float32)
    w2 = np.asarray(w2, dtype=np.float32)
    w3 = np.asarray(w3, dtype=np.float32)
    mid_w = np.asarray(mid_w, dtype=np.float32)
    counts = np.asarray(num_tokens_per_expert).astype(np.int64)

    T_, D_ = x.shape
    E_, F_, _ = w1.shape
    Ccap = (T_ // E_) * 3 // 2  # reference static capacity (768)
    ends = np.cumsum(counts)
    starts = ends - counts
    eff = np.minimum(np.maximum(counts, 0), Ccap)  # rows actually computed

    C = int(max(P, -(-int(eff.max()) // P) * P))  # pad to token-tile multiple
    nc = _get_program(C)

    KD = D_ // P
    KF = F_ // P
    FB = F_ // NB
    DB = D_ // NB

    in_maps = []
    for e in range(E_):
        cnt = int(eff[e])
        s = int(starts[e])
        xg = np.zeros((C, D_), np.float32)
        if cnt > 0:
            rows = np.clip(s + np.arange(cnt), 0, T_ - 1)
            xg[:cnt] = x[rows]
        # [P, NT, KD, P] token-tile-major; every DMA slab contiguous.
        xg4 = xg.astype(np.float16).reshape(C // P, P, KD, P)
        w1p = w1[e].T.astype(np.float16).reshape(KD, P, FB, NB)
        w3p = w3[e].T.astype(np.float16).reshape(KD, P, FB, NB)
        w2p = (w2[e] * mid_w[None, :]).T.astype(np.float16).reshape(KF, P, DB, NB)
        in_maps.append(
            {
                "xT": np.ascontiguousarray(xg4.transpose(3, 0, 2, 1)),
                "w1t": np.ascontiguousarray(w1p.transpose(1, 2, 0, 3)),
                "w3t": np.ascontiguousarray(w3p.transpose(1, 2, 0, 3)),
                "w2t": np.ascontiguousarray(w2p.transpose(1, 2, 0, 3)),
            }
        )

    LAST_RESULTS = _run(nc, in_maps)
    outs = [LAST_RESULTS[e]["out"] for e in range(E_)]

    # scatter back to flat token order, mirroring the reference's clamping
    tok = np.arange(T_)
    eid = np.clip(np.searchsorted(ends, tok, side="right"), 0, E_ - 1)
    pos = tok - starts[eid]
    idx = np.minimum(pos, Ccap - 1)
    valid = (idx >= 0) & (idx < eff[eid])
    idx_safe = np.clip(idx, 0, C - 1)
    stacked = np.stack(outs, axis=0)  # [E, C, D]
    result = stacked[eid, idx_safe].astype(np.float32)
    result[~valid] = 0.0
    return result



# revision 3
# speedup vs baseline: 1.0050x; 1.0050x over previous
"""Trainium2 Bass kernel: grouped-experts SwiGLU MLP with mid-RMSNorm.

Expert-parallel across 8 NeuronCores: core e computes expert e's token
block (tokens are pre-sorted by expert).

v2: weight-stationary ("flipped") matmul orientation.  The moving
operand is the token axis, so PE cost scales with the actual padded
token count (576, vs 640 tile-padded in v1) and the SwiGLU hidden state
is produced directly in [f, tok] layout -- exactly what the down-proj
matmul consumes -- eliminating all PE transposes.  The mid-RMSNorm
row scale commutes with the (linear) down projection, so the device
only produces ssq[tok] = sum_f h^2 (via a ones-column matmul) and the
host applies rstd; mid_w is folded into w2.

Per-core math (fp16 in / fp32 accumulate), NTOK = 576 padded tokens:
    h1[f,t] = sum_dk w1c[dk].T @ x[dk]     (16 accum MMs per f-chunk)
    h  = silu(h1) * h3                     # [128, KF, NTOK] in SBUF
    ssq[t] = ones.T @ (h*h)                # [1, NTOK] accum over KF
    out[d,t] = sum_fk w2c[fk].T @ h[fk]    # [KD, 128, NTOK]
Host: out_rows *= rsqrt(ssq/F + eps); scatter to flat token order.

PSUM: one pool, 4 tags x full 2KB bank x 2 bufs = 8 banks exactly; no
two accumulation tiles share a bank, so PE-write never collides with
ScalarE/VectorE reads of a neighbouring tile.

DMA: w1 on sync, w3 on gpsimd, x + w2 on scalar (w2 is only needed for
phase C at ~2/3 of the kernel), outputs alternate sync/gpsimd.  First
slabs of x and w1/w3 are split small so the first matmuls gate early;
~12 warm-up matmuls cover the initial DMA wait and HAM ramp.
"""

import sys

sys.path.insert(0, "/opt/trn_rl_repo")

import os

import numpy as np
from contextlib import ExitStack

import concourse.bass as bass
import concourse.tile as tile
from concourse import bacc, mybir

P = 128
D = 2048
F = 1024
E = 8
KD = D // P  # 16 contraction chunks for mm1/mm3
KF = F // P  # 8 f chunks (contraction chunks for mm2)
EPS = 1e-6
F32 = mybir.dt.float32
F16 = mybir.dt.float16
ACTF = mybir.ActivationFunctionType

_PROGRAM_CACHE: dict[int, object] = {}
LAST_RESULTS = None  # test harness reads per-core outputs from here


def _run(nc, in_maps):
    """Execute the compiled program on the 8 axon-tunneled cores.

    If KERNEL_NTFF_DIR is set, wrap the execute in the axon NTFF profile
    hook so device profiles land there (test harness use only).
    """
    from concourse import bass2jax

    ntff_dir = os.environ.get("KERNEL_NTFF_DIR")
    if ntff_dir:
        if "/root/.axon_site" not in sys.path:
            sys.path.insert(0, "/root/.axon_site")
        from trn_agent_boot.trn_boot import _ntff_profile_via_ctypes

        hook = _ntff_profile_via_ctypes("/opt/axon/libaxon_pjrt.so")
        ids = [
            int(x) for x in os.environ.get("KERNEL_NTFF_CORES", "0").split(",")
        ]
        if hook is not None:
            with hook(ntff_dir, ids):
                return bass2jax.run_bass_via_pjrt(nc, in_maps, n_cores=len(in_maps))
    return bass2jax.run_bass_via_pjrt(nc, in_maps, n_cores=len(in_maps))


def _build_program(NTOK: int):
    """Build + compile the single-core SPMD program for NTOK padded tokens."""
    # token chunks, each <= 512 (one fp32 PSUM bank of moving dim)
    CH = [(0, min(512, NTOK))]
    if NTOK > 512:
        CH.append((512, NTOK))
    NCH = len(CH)

    nc = bacc.Bacc(
        "TRN2",
        target_bir_lowering=False,
        debug=False,
        enable_asserts=False,
        num_devices=E,
    )
    xT_d = nc.dram_tensor("xT", [P, KD, NTOK], F16, kind="ExternalInput").ap()
    w1_d = nc.dram_tensor("w1t", [P, KF, KD, P], F16, kind="ExternalInput").ap()
    w3_d = nc.dram_tensor("w3t", [P, KF, KD, P], F16, kind="ExternalInput").ap()
    w2_d = nc.dram_tensor("w2t", [P, KD, KF, P], F16, kind="ExternalInput").ap()
    out_d = nc.dram_tensor("out", [KD, P, NTOK], F16, kind="ExternalOutput").ap()
    ssq_d = nc.dram_tensor("ssq", [1, NTOK], F32, kind="ExternalOutput").ap()

    with tile.TileContext(nc) as tc, ExitStack() as ctx:
        singles = ctx.enter_context(tc.tile_pool(name="singles", bufs=1))
        xpool = ctx.enter_context(tc.tile_pool(name="x", bufs=1))
        w1pool = ctx.enter_context(tc.tile_pool(name="w1", bufs=1))
        w3pool = ctx.enter_context(tc.tile_pool(name="w3", bufs=1))
        w2pool = ctx.enter_context(tc.tile_pool(name="w2", bufs=1))
        hpool = ctx.enter_context(tc.tile_pool(name="h", bufs=1))
        qpool = ctx.enter_context(tc.tile_pool(name="hsq", bufs=1))
        spool = ctx.enter_context(tc.tile_pool(name="scr", bufs=2))
        opool = ctx.enter_context(tc.tile_pool(name="o", bufs=4))
        psp = ctx.enter_context(tc.tile_pool(name="ps", bufs=2, space="PSUM"))

        warm = singles.tile([P, 512], F16, name="warm")
        nc.gpsimd.memset(warm[:], 0.5)
        ones = singles.tile([P, 1], F16, name="ones")
        nc.gpsimd.memset(ones[:], 1.0)
        ssq_sb = singles.tile([1, NTOK], F32, name="ssq_sb")

        xsb = xpool.tile([P, KD, NTOK], F16)
        w1sb = w1pool.tile([P, KF, KD, P], F16)
        w3sb = w3pool.tile([P, KF, KD, P], F16)
        w2sb = w2pool.tile([P, KD, KF, P], F16)
        h = hpool.tile([P, KF, NTOK], F16)
        hsq = qpool.tile([P, KF, NTOK], F16)

        # ---- DMA schedule (consumption order; 3 parallel queues) --------
        # scalar: x first (gates everything), then w2 (needed at phase C).
        nc.scalar.dma_start(xsb[:, 0:2, :], xT_d[:, 0:2, :])
        nc.scalar.dma_start(xsb[:, 2:6, :], xT_d[:, 2:6, :])
        nc.scalar.dma_start(xsb[:, 6:11, :], xT_d[:, 6:11, :])
        nc.scalar.dma_start(xsb[:, 11:KD, :], xT_d[:, 11:KD, :])
        # sync: w1 fk-slabs; gpsimd: w3 fk-slabs (fk0 split for early gate)
        nc.sync.dma_start(w1sb[:, 0, 0:4, :], w1_d[:, 0, 0:4, :])
        nc.gpsimd.dma_start(w3sb[:, 0, 0:4, :], w3_d[:, 0, 0:4, :])
        nc.sync.dma_start(w1sb[:, 0, 4:KD, :], w1_d[:, 0, 4:KD, :])
        nc.gpsimd.dma_start(w3sb[:, 0, 4:KD, :], w3_d[:, 0, 4:KD, :])
        for fk in range(1, KF):
            nc.sync.dma_start(w1sb[:, fk], w1_d[:, fk])
            nc.gpsimd.dma_start(w3sb[:, fk], w3_d[:, fk])
        for dk in range(0, KD, 2):
            nc.scalar.dma_start(w2sb[:, dk : dk + 2], w2_d[:, dk : dk + 2])

        # HAM warm-up: keep the PE busy through the prologue DMA wait so
        # the clock gate is at 8/8 when the first real matmul's data lands.
        ps_w = psp.tile([P, 512], F32, tag="pa0", name="warm_ps")
        for _ in range(12):
            nc.tensor.matmul(ps_w[:], warm[:, 0:P], warm[:], start=True, stop=True)

        # ================= phase A: h = silu(x@w1^T) * (x@w3^T) ===========
        for fk in range(KF):
            p1 = [psp.tile([P, 512], F32, tag=f"pa{i}", name=f"p1_{fk}_{i}") for i in range(NCH)]
            p3 = [psp.tile([P, 512], F32, tag=f"pc{i}", name=f"p3_{fk}_{i}") for i in range(NCH)]
            for dk in range(KD):
                wc1 = w1sb[:, fk, dk, :]
                wc3 = w3sb[:, fk, dk, :]
                st = dk == 0
                sp = dk == KD - 1
                for i, (a, b) in enumerate(CH):
                    nc.tensor.matmul(
                        p1[i][:, 0 : b - a], wc1, xsb[:, dk, a:b], start=st, stop=sp
                    )
                for i, (a, b) in enumerate(CH):
                    nc.tensor.matmul(
                        p3[i][:, 0 : b - a], wc3, xsb[:, dk, a:b], start=st, stop=sp
                    )
            for i, (a, b) in enumerate(CH):
                w = b - a
                s = spool.tile([P, 512], F32, tag=f"sig{i}", name=f"sig_{fk}_{i}")
                nc.scalar.activation(s[:, 0:w], p1[i][:, 0:w], ACTF.Sigmoid)
                hs = h[:, fk, a:b]
                nc.vector.tensor_mul(hs, s[:, 0:w], p1[i][:, 0:w])
                nc.vector.tensor_mul(hs, hs, p3[i][:, 0:w])
                nc.vector.tensor_mul(hsq[:, fk, a:b], hs, hs)

        # ================= ssq[t] = sum_f h^2 (ones-column matmuls) =======
        sacc = [psp.tile([P, 512], F32, tag=f"pc{i}", name=f"sacc{i}") for i in range(NCH)]
        for fk in range(KF):
            for i, (a, b) in enumerate(CH):
                nc.tensor.matmul(
                    sacc[i][0:1, 0 : b - a],
                    ones[:],
                    hsq[:, fk, a:b],
                    start=(fk == 0),
                    stop=(fk == KF - 1),
                )
        for i, (a, b) in enumerate(CH):
            nc.vector.tensor_copy(ssq_sb[:, a:b], sacc[i][0:1, 0 : b - a])
        nc.scalar.dma_start(ssq_d[:, :], ssq_sb[:])

        # ================= phase C: out[d,t] = sum_fk w2c.T @ h ===========
        for dk in range(KD):
            po = [psp.tile([P, 512], F32, tag=f"pa{i}", name=f"po_{dk}_{i}") for i in range(NCH)]
            for fk in range(KF):
                wc2 = w2sb[:, dk, fk, :]
                st = fk == 0
                sp = fk == KF - 1
                for i, (a, b) in enumerate(CH):
                    nc.tensor.matmul(
                        po[i][:, 0 : b - a], wc2, h[:, fk, a:b], start=st, stop=sp
                    )
            ob = opool.tile([P, NTOK], F16, tag="ob")
            for i, (a, b) in enumerate(CH):
                nc.vector.tensor_copy(ob[:, a:b], po[i][:, 0 : b - a])
            oq = nc.sync if dk % 2 == 0 else nc.gpsimd
            oq.dma_start(out_d[dk], ob[:])

    nc.compile()
    return nc


def _get_program(NTOK: int):
    if NTOK not in _PROGRAM_CACHE:
        _PROGRAM_CACHE[NTOK] = _build_program(NTOK)
    return _PROGRAM_CACHE[NTOK]


def kernel(x, w1, w2, w3, mid_w, num_tokens_per_expert):
    global LAST_RESULTS
    x = np.ascontiguousarray(np.asarray(x, dtype=np.float32))
    w1 = np.asarray(w1, dtype=np.float32)
    w2 = np.asarray(w2, dtype=np.float32)
    w3 = np.asarray(w3, dtype=np.float32)
    mid_w = np.asarray(mid_w, dtype=np.float32)
    counts = np.asarray(num_tokens_per_expert).astype(np.int64)

    T_, D_ = x.shape
    E_, F_, _ = w1.shape
    Ccap = (T_ // E_) * 3 // 2  # reference static capacity (768)
    ends = np.cumsum(counts)
    starts = ends - counts
    eff = np.minimum(np.maximum(counts, 0), Ccap)  # rows actually computed

    NTOK = int(max(64, -(-int(eff.max()) // 64) * 64))  # pad to 64 tokens
    nc = _get_program(NTOK)

    in_maps = []
    for e in range(E_):
        cnt = int(eff[e])
        s = int(starts[e])
        xg = np.zeros((NTOK, D_), np.float32)
        if cnt > 0:
            rows = np.clip(s + np.arange(cnt), 0, T_ - 1)
            xg[:cnt] = x[rows]
        # xT: [P(p), KD, NTOK] with [p, dk, t] = x[t, dk*128+p]
        xT = xg.T.astype(np.float16).reshape(KD, P, NTOK).transpose(1, 0, 2)
        # w1t/w3t: [p, fk, dk, q] = w[fk*128+q, dk*128+p]   (w is [F, D])
        w1t = w1[e].astype(np.float16).reshape(KF, P, KD, P).transpose(3, 0, 2, 1)
        w3t = w3[e].astype(np.float16).reshape(KF, P, KD, P).transpose(3, 0, 2, 1)
        # w2t: [p, dk, fk, q] = w2m[dk*128+q, fk*128+p]     (w2m is [D, F])
        w2m = w2[e] * mid_w[None, :]
        w2t = w2m.astype(np.float16).reshape(KD, P, KF, P).transpose(3, 0, 2, 1)
        in_maps.append(
            {
                "xT": np.ascontiguousarray(xT),
                "w1t": np.ascontiguousarray(w1t),
                "w3t": np.ascontiguousarray(w3t),
                "w2t": np.ascontiguousarray(w2t),
            }
        )

    LAST_RESULTS = _run(nc, in_maps)

    outs = []
    for e in range(E_):
        o = np.asarray(LAST_RESULTS[e]["out"], np.float32).reshape(D_, NTOK).T
        ssq = np.asarray(LAST_RESULTS[e]["ssq"], np.float32).reshape(NTOK)
        rstd = 1.0 / np.sqrt(ssq / F_ + EPS)
        outs.append(o * rstd[:, None])  # [NTOK, D]
    stacked = np.stack(outs, axis=0)  # [E, NTOK, D]

    # scatter back to flat token order, mirroring the reference's clamping
    tok = np.arange(T_)
    eid = np.clip(np.searchsorted(ends, tok, side="right"), 0, E_ - 1)
    pos = tok - starts[eid]
    idx = np.minimum(pos, Ccap - 1)
    valid = (idx >= 0) & (idx < eff[eid])
    idx_safe = np.clip(idx, 0, NTOK - 1)
    result = stacked[eid, idx_safe].astype(np.float32)
    result[~valid] = 0.0
    return result


# revision 6
# speedup vs baseline: 1.0820x; 1.0767x over previous
"""Trainium2 Bass kernel: grouped-experts SwiGLU MLP with mid-RMSNorm.

Expert-parallel across 8 NeuronCores: core e computes expert e's token
block (tokens are pre-sorted by expert).

v2: weight-stationary ("flipped") matmul orientation.  The moving
operand is the token axis, so PE cost scales with the actual padded
token count (576, vs 640 tile-padded in v1) and the SwiGLU hidden state
is produced directly in [f, tok] layout -- exactly what the down-proj
matmul consumes -- eliminating all PE transposes.  The mid-RMSNorm
row scale commutes with the (linear) down projection, so the device
only produces ssq[tok] = sum_f h^2 (via a ones-column matmul) and the
host applies rstd; mid_w is folded into w2.

Per-core math (fp16 in / fp32 accumulate), NTOK = 576 padded tokens:
    h1[f,t] = sum_dk w1c[dk].T @ x[dk]     (16 accum MMs per f-chunk)
    h  = silu(h1) * h3                     # [128, KF, NTOK] in SBUF
    ssq[t] = ones.T @ (h*h)                # [1, NTOK] accum over KF
    out[d,t] = sum_fk w2c[fk].T @ h[fk]    # [KD, 128, NTOK]
Host: out_rows *= rsqrt(ssq/F + eps); scatter to flat token order.

PSUM: one pool, 4 tags x full 2KB bank x 2 bufs = 8 banks exactly; no
two accumulation tiles share a bank, so PE-write never collides with
ScalarE/VectorE reads of a neighbouring tile.

DMA: w1 on sync, w3 on gpsimd, x + w2 on scalar (w2 is only needed for
phase C at ~2/3 of the kernel), outputs alternate sync/gpsimd.  First
slabs of x and w1/w3 are split small so the first matmuls gate early;
~12 warm-up matmuls cover the initial DMA wait and HAM ramp.
"""

import sys

sys.path.insert(0, "/opt/trn_rl_repo")

import os

import numpy as np
from contextlib import ExitStack

import concourse.bass as bass
import concourse.tile as tile
from concourse import bacc, mybir

P = 128
D = 2048
F = 1024
E = 8
KD = D // P  # 16 contraction chunks for mm1/mm3
KF = F // P  # 8 f chunks (contraction chunks for mm2)
EPS = 1e-6
F32 = mybir.dt.float32
F16 = mybir.dt.float16
ACTF = mybir.ActivationFunctionType

_PROGRAM_CACHE: dict[int, object] = {}
LAST_RESULTS = None  # test harness reads per-core outputs from here


def _run(nc, in_maps):
    """Execute the compiled program on the 8 axon-tunneled cores.

    If KERNEL_NTFF_DIR is set, wrap the execute in the axon NTFF profile
    hook so device profiles land there (test harness use only).
    """
    from concourse import bass2jax

    ntff_dir = os.environ.get("KERNEL_NTFF_DIR")
    if ntff_dir:
        if "/root/.axon_site" not in sys.path:
            sys.path.insert(0, "/root/.axon_site")
        from trn_agent_boot.trn_boot import _ntff_profile_via_ctypes

        hook = _ntff_profile_via_ctypes("/opt/axon/libaxon_pjrt.so")
        ids = [
            int(x) for x in os.environ.get("KERNEL_NTFF_CORES", "0").split(",")
        ]
        if hook is not None:
            with hook(ntff_dir, ids):
                return bass2jax.run_bass_via_pjrt(nc, in_maps, n_cores=len(in_maps))
    return bass2jax.run_bass_via_pjrt(nc, in_maps, n_cores=len(in_maps))


def _build_program(NTOK: int):
    """Build + compile the single-core SPMD program for NTOK padded tokens."""
    # token chunks, each <= 512 (one fp32 PSUM bank of moving dim)
    CH = [(0, min(512, NTOK))]
    if NTOK > 512:
        CH.append((512, NTOK))
    NCH = len(CH)

    nc = bacc.Bacc(
        "TRN2",
        target_bir_lowering=False,
        debug=False,
        enable_asserts=False,
        num_devices=E,
    )
    xT_d = nc.dram_tensor("xT", [P, KD, NTOK], F16, kind="ExternalInput").ap()
    w1_d = nc.dram_tensor("w1t", [P, KF, KD, P], F16, kind="ExternalInput").ap()
    w3_d = nc.dram_tensor("w3t", [P, KF, KD, P], F16, kind="ExternalInput").ap()
    w2_d = nc.dram_tensor("w2t", [P, KD, KF, P], F16, kind="ExternalInput").ap()
    out_d = nc.dram_tensor("out", [P, KD, NTOK], F16, kind="ExternalOutput").ap()
    ssq_d = nc.dram_tensor("ssq", [1, NTOK], F32, kind="ExternalOutput").ap()

    with tile.TileContext(nc) as tc, ExitStack() as ctx:
        singles = ctx.enter_context(tc.tile_pool(name="singles", bufs=1))
        xpool = ctx.enter_context(tc.tile_pool(name="x", bufs=1))
        w1pool = ctx.enter_context(tc.tile_pool(name="w1", bufs=1))
        w3pool = ctx.enter_context(tc.tile_pool(name="w3", bufs=1))
        w2pool = ctx.enter_context(tc.tile_pool(name="w2", bufs=1))
        hpool = ctx.enter_context(tc.tile_pool(name="h", bufs=1))
        qpool = ctx.enter_context(tc.tile_pool(name="hsq", bufs=1))
        spool = ctx.enter_context(tc.tile_pool(name="scr", bufs=2))
        opool = ctx.enter_context(tc.tile_pool(name="o", bufs=1))
        psp = ctx.enter_context(tc.tile_pool(name="ps", bufs=2, space="PSUM"))

        warm = singles.tile([P, 512], F16, name="warm")
        nc.gpsimd.memset(warm[:], 0.5)
        ones = singles.tile([P, 1], F16, name="ones")
        nc.gpsimd.memset(ones[:], 1.0)
        ssq_sb = singles.tile([1, NTOK], F32, name="ssq_sb")

        xsb = xpool.tile([P, KD, NTOK], F16)
        w1sb = w1pool.tile([P, KF, KD, P], F16)
        w3sb = w3pool.tile([P, KF, KD, P], F16)
        w2sb = w2pool.tile([P, KD, KF, P], F16)
        h = hpool.tile([P, KF, NTOK], F16)
        hsq = qpool.tile([P, KF, NTOK], F16)

        # ---- DMA schedule (consumption order; 3 parallel queues) --------
        # Startup-critical bytes: all of x (re-read per fk from SBUF, so the
        # full 2.4MB gates fk0's last dk chunks) + fk0 weights.  Spread x
        # over all three queues; w2 rides sync/gpsimd AFTER w1/w3 (never on
        # scalar: the ACT queue is FIFO and must stay free for phase-A
        # sigmoids, which release PSUM).
        nc.scalar.dma_start(xsb[:, 0:2, :], xT_d[:, 0:2, :])
        nc.sync.dma_start(w1sb[:, 0, 0:4, :], w1_d[:, 0, 0:4, :])
        nc.gpsimd.dma_start(w3sb[:, 0, 0:4, :], w3_d[:, 0, 0:4, :])
        nc.scalar.dma_start(xsb[:, 2:4, :], xT_d[:, 2:4, :])
        nc.sync.dma_start(w1sb[:, 0, 4:KD, :], w1_d[:, 0, 4:KD, :])
        nc.gpsimd.dma_start(w3sb[:, 0, 4:KD, :], w3_d[:, 0, 4:KD, :])
        nc.scalar.dma_start(xsb[:, 4:7, :], xT_d[:, 4:7, :])
        nc.sync.dma_start(xsb[:, 10:13, :], xT_d[:, 10:13, :])
        nc.gpsimd.dma_start(xsb[:, 13:KD, :], xT_d[:, 13:KD, :])
        nc.scalar.dma_start(xsb[:, 7:10, :], xT_d[:, 7:10, :])
        for fk in range(1, KF):
            nc.sync.dma_start(w1sb[:, fk], w1_d[:, fk])
            nc.gpsimd.dma_start(w3sb[:, fk], w3_d[:, fk])
        for dk in range(0, KD, 2):
            q = nc.sync if (dk // 2) % 2 == 0 else nc.gpsimd
            q.dma_start(w2sb[:, dk : dk + 2], w2_d[:, dk : dk + 2])

        # HAM warm-up: keep the PE busy through the prologue DMA wait so
        # the clock gate is at 8/8 when the first real matmul's data lands.
        ps_w = psp.tile([P, 512], F32, tag="pa0", name="warm_ps")
        for _ in range(12):
            nc.tensor.matmul(ps_w[:], warm[:, 0:P], warm[:], start=True, stop=True)

        # ================= phase A: h = silu(x@w1^T) * (x@w3^T) ===========
        for fk in range(KF):
            p1 = [psp.tile([P, 512], F32, tag=f"pa{i}", name=f"p1_{fk}_{i}") for i in range(NCH)]
            p3 = [psp.tile([P, 512], F32, tag=f"pc{i}", name=f"p3_{fk}_{i}") for i in range(NCH)]
            for dk in range(KD):
                wc1 = w1sb[:, fk, dk, :]
                wc3 = w3sb[:, fk, dk, :]
                st = dk == 0
                sp = dk == KD - 1
                for i, (a, b) in enumerate(CH):
                    nc.tensor.matmul(
                        p1[i][:, 0 : b - a], wc1, xsb[:, dk, a:b], start=st, stop=sp
                    )
                for i, (a, b) in enumerate(CH):
                    nc.tensor.matmul(
                        p3[i][:, 0 : b - a], wc3, xsb[:, dk, a:b], start=st, stop=sp
                    )
            for i, (a, b) in enumerate(CH):
                w = b - a
                s = spool.tile([P, 512], F32, tag=f"sig{i}", name=f"sig_{fk}_{i}")
                nc.scalar.activation(s[:, 0:w], p1[i][:, 0:w], ACTF.Sigmoid)
                hs = h[:, fk, a:b]
                nc.vector.tensor_mul(hs, s[:, 0:w], p1[i][:, 0:w])
                nc.vector.tensor_mul(hs, hs, p3[i][:, 0:w])
                nc.vector.tensor_mul(hsq[:, fk, a:b], hs, hs)

        # ================= ssq[t] = sum_f h^2 (ones-column matmuls) =======
        sacc = [psp.tile([P, 512], F32, tag=f"pc{i}", name=f"sacc{i}") for i in range(NCH)]
        for fk in range(KF):
            for i, (a, b) in enumerate(CH):
                nc.tensor.matmul(
                    sacc[i][0:1, 0 : b - a],
                    ones[:],
                    hsq[:, fk, a:b],
                    start=(fk == 0),
                    stop=(fk == KF - 1),
                )
        for i, (a, b) in enumerate(CH):
            nc.vector.tensor_copy(ssq_sb[:, a:b], sacc[i][0:1, 0 : b - a])
        nc.scalar.dma_start(ssq_d[:, :], ssq_sb[:])

        # ================= phase C: out[d,t] = sum_fk w2c.T @ h ===========
        # output batches: big batches overlap phase C; tiny last batch so the
        # kernel end is not gated on a large transfer + completion receipt.
        OBATCH = [(0, 6), (6, 12), (12, 15), (15, 16)]
        obt = {
            g0: opool.tile([P, g1 - g0, NTOK], F16, tag=f"ob{g0}", name=f"ob{g0}")
            for g0, g1 in OBATCH
        }
        for gi, (g0, g1) in enumerate(OBATCH):
            ob = obt[g0]
            for dk in range(g0, g1):
                po = [psp.tile([P, 512], F32, tag=f"pa{i}", name=f"po_{dk}_{i}") for i in range(NCH)]
                for fk in range(KF):
                    wc2 = w2sb[:, dk, fk, :]
                    st = fk == 0
                    sp = fk == KF - 1
                    for i, (a, b) in enumerate(CH):
                        nc.tensor.matmul(
                            po[i][:, 0 : b - a], wc2, h[:, fk, a:b], start=st, stop=sp
                        )
                for i, (a, b) in enumerate(CH):
                    nc.vector.tensor_copy(ob[:, dk - g0, a:b], po[i][:, 0 : b - a])
            oq = nc.scalar if gi % 2 == 0 else nc.sync
            oq.dma_start(out_d[:, g0:g1, :], ob[:])

    nc.compile()
    return nc


def _get_program(NTOK: int):
    if NTOK not in _PROGRAM_CACHE:
        _PROGRAM_CACHE[NTOK] = _build_program(NTOK)
    return _PROGRAM_CACHE[NTOK]


def kernel(x, w1, w2, w3, mid_w, num_tokens_per_expert):
    global LAST_RESULTS
    x = np.ascontiguousarray(np.asarray(x, dtype=np.float32))
    w1 = np.asarray(w1, dtype=np.float32)
    w2 = np.asarray(w2, dtype=np.float32)
    w3 = np.asarray(w3, dtype=np.float32)
    mid_w = np.asarray(mid_w, dtype=np.float32)
    counts = np.asarray(num_tokens_per_expert).astype(np.int64)

    T_, D_ = x.shape
    E_, F_, _ = w1.shape
    Ccap = (T_ // E_) * 3 // 2  # reference static capacity (768)
    ends = np.cumsum(counts)
    starts = ends - counts
    eff = np.minimum(np.maximum(counts, 0), Ccap)  # rows actually computed

    NTOK = int(max(64, -(-int(eff.max()) // 64) * 64))  # pad to 64 tokens
    nc = _get_program(NTOK)

    in_maps = []
    for e in range(E_):
        cnt = int(eff[e])
        s = int(starts[e])
        xg = np.zeros((NTOK, D_), np.float32)
        if cnt > 0:
            rows = np.clip(s + np.arange(cnt), 0, T_ - 1)
            xg[:cnt] = x[rows]
        # xT: [P(p), KD, NTOK] with [p, dk, t] = x[t, dk*128+p]
        xT = xg.T.astype(np.float16).reshape(KD, P, NTOK).transpose(1, 0, 2)
        # w1t/w3t: [p, fk, dk, q] = w[fk*128+q, dk*128+p]   (w is [F, D])
        w1t = w1[e].astype(np.float16).reshape(KF, P, KD, P).transpose(3, 0, 2, 1)
        w3t = w3[e].astype(np.float16).reshape(KF, P, KD, P).transpose(3, 0, 2, 1)
        # w2t: [p, dk, fk, q] = w2m[dk*128+q, fk*128+p]     (w2m is [D, F])
        w2m = w2[e] * mid_w[None, :]
        w2t = w2m.astype(np.float16).reshape(KD, P, KF, P).transpose(3, 0, 2, 1)
        in_maps.append(
            {
                "xT": np.ascontiguousarray(xT),
                "w1t": np.ascontiguousarray(w1t),
                "w3t": np.ascontiguousarray(w3t),
                "w2t": np.ascontiguousarray(w2t),
            }
        )

    LAST_RESULTS = _run(nc, in_maps)

    outs = []
    for e in range(E_):
        o = (
            np.asarray(LAST_RESULTS[e]["out"], np.float32)
            .transpose(1, 0, 2)
            .reshape(D_, NTOK)
            .T
        )
        ssq = np.asarray(LAST_RESULTS[e]["ssq"], np.float32).reshape(NTOK)
        rstd = 1.0 / np.sqrt(ssq / F_ + EPS)
        outs.append(o * rstd[:, None])  # [NTOK, D]
    stacked = np.stack(outs, axis=0)  # [E, NTOK, D]

    # scatter back to flat token order, mirroring the reference's clamping
    tok = np.arange(T_)
    eid = np.clip(np.searchsorted(ends, tok, side="right"), 0, E_ - 1)
    pos = tok - starts[eid]
    idx = np.minimum(pos, Ccap - 1)
    valid = (idx >= 0) & (idx < eff[eid])
    idx_safe = np.clip(idx, 0, NTOK - 1)
    result = stacked[eid, idx_safe].astype(np.float32)
    result[~valid] = 0.0
    return result


# revision 7
# speedup vs baseline: 1.0887x; 1.0062x over previous
"""Trainium2 Bass kernel: grouped-experts SwiGLU MLP with mid-RMSNorm.

Expert-parallel across 8 NeuronCores: core e computes expert e's token
block (tokens are pre-sorted by expert).

v2: weight-stationary ("flipped") matmul orientation.  The moving
operand is the token axis, so PE cost scales with the actual padded
token count (576, vs 640 tile-padded in v1) and the SwiGLU hidden state
is produced directly in [f, tok] layout -- exactly what the down-proj
matmul consumes -- eliminating all PE transposes.  The mid-RMSNorm
row scale commutes with the (linear) down projection, so the device
only produces ssq[tok] = sum_f h^2 (via a ones-column matmul) and the
host applies rstd; mid_w is folded into w2.

Per-core math (fp16 in / fp32 accumulate), NTOK = 576 padded tokens:
    h1[f,t] = sum_dk w1c[dk].T @ x[dk]     (16 accum MMs per f-chunk)
    h  = silu(h1) * h3                     # [128, KF, NTOK] in SBUF
    ssq[t] = ones.T @ (h*h)                # [1, NTOK] accum over KF
    out[d,t] = sum_fk w2c[fk].T @ h[fk]    # [KD, 128, NTOK]
Host: out_rows *= rsqrt(ssq/F + eps); scatter to flat token order.

PSUM: one pool, 4 tags x full 2KB bank x 2 bufs = 8 banks exactly; no
two accumulation tiles share a bank, so PE-write never collides with
ScalarE/VectorE reads of a neighbouring tile.

DMA: w1 on sync, w3 on gpsimd, x + w2 on scalar (w2 is only needed for
phase C at ~2/3 of the kernel), outputs alternate sync/gpsimd.  First
slabs of x and w1/w3 are split small so the first matmuls gate early;
~12 warm-up matmuls cover the initial DMA wait and HAM ramp.
"""

import sys

sys.path.insert(0, "/opt/trn_rl_repo")

import os

import numpy as np
from contextlib import ExitStack

import concourse.bass as bass
import concourse.tile as tile
from concourse import bacc, mybir

P = 128
D = 2048
F = 1024
E = 8
KD = D // P  # 16 contraction chunks for mm1/mm3
KF = F // P  # 8 f chunks (contraction chunks for mm2)
EPS = 1e-6
F32 = mybir.dt.float32
F16 = mybir.dt.float16
ACTF = mybir.ActivationFunctionType

_PROGRAM_CACHE: dict[int, object] = {}
LAST_RESULTS = None  # test harness reads per-core outputs from here


def _run(nc, in_maps):
    """Execute the compiled program on the 8 axon-tunneled cores.

    If KERNEL_NTFF_DIR is set, wrap the execute in the axon NTFF profile
    hook so device profiles land there (test harness use only).
    """
    from concourse import bass2jax

    ntff_dir = os.environ.get("KERNEL_NTFF_DIR")
    if ntff_dir:
        if "/root/.axon_site" not in sys.path:
            sys.path.insert(0, "/root/.axon_site")
        from trn_agent_boot.trn_boot import _ntff_profile_via_ctypes

        hook = _ntff_profile_via_ctypes("/opt/axon/libaxon_pjrt.so")
        ids = [
            int(x) for x in os.environ.get("KERNEL_NTFF_CORES", "0").split(",")
        ]
        if hook is not None:
            with hook(ntff_dir, ids):
                return bass2jax.run_bass_via_pjrt(nc, in_maps, n_cores=len(in_maps))
    return bass2jax.run_bass_via_pjrt(nc, in_maps, n_cores=len(in_maps))


def _build_program(NTOK: int):
    """Build + compile the single-core SPMD program for NTOK padded tokens."""
    # token chunks, each <= 512 (one fp32 PSUM bank of moving dim)
    CH = [(0, min(512, NTOK))]
    if NTOK > 512:
        CH.append((512, NTOK))
    NCH = len(CH)

    nc = bacc.Bacc(
        "TRN2",
        target_bir_lowering=False,
        debug=False,
        enable_asserts=False,
        num_devices=E,
    )
    xT_d = nc.dram_tensor("xT", [P, KD, NTOK], F16, kind="ExternalInput").ap()
    w1_d = nc.dram_tensor("w1t", [P, KF, KD, P], F16, kind="ExternalInput").ap()
    w3_d = nc.dram_tensor("w3t", [P, KF, KD, P], F16, kind="ExternalInput").ap()
    w2_d = nc.dram_tensor("w2t", [P, KD, KF, P], F16, kind="ExternalInput").ap()
    out_d = nc.dram_tensor("out", [P, KD, NTOK], F16, kind="ExternalOutput").ap()
    ssq_d = nc.dram_tensor("ssq", [1, NTOK], F32, kind="ExternalOutput").ap()

    with tile.TileContext(nc) as tc, ExitStack() as ctx:
        singles = ctx.enter_context(tc.tile_pool(name="singles", bufs=1))
        xpool = ctx.enter_context(tc.tile_pool(name="x", bufs=1))
        w1pool = ctx.enter_context(tc.tile_pool(name="w1", bufs=1))
        w3pool = ctx.enter_context(tc.tile_pool(name="w3", bufs=1))
        w2pool = ctx.enter_context(tc.tile_pool(name="w2", bufs=1))
        hpool = ctx.enter_context(tc.tile_pool(name="h", bufs=1))
        qpool = ctx.enter_context(tc.tile_pool(name="hsq", bufs=1))
        spool = ctx.enter_context(tc.tile_pool(name="scr", bufs=2))
        opool = ctx.enter_context(tc.tile_pool(name="o", bufs=1))
        psp = ctx.enter_context(tc.tile_pool(name="ps", bufs=2, space="PSUM"))

        warm = singles.tile([P, 512], F16, name="warm")
        nc.gpsimd.memset(warm[:], 0.5)
        ones = singles.tile([P, 1], F16, name="ones")
        nc.gpsimd.memset(ones[:], 1.0)
        ssq_sb = singles.tile([1, NTOK], F32, name="ssq_sb")

        xsb = xpool.tile([P, KD, NTOK], F16)
        w1sb = w1pool.tile([P, KF, KD, P], F16)
        w3sb = w3pool.tile([P, KF, KD, P], F16)
        w2sb = w2pool.tile([P, KD, KF, P], F16)
        h = hpool.tile([P, KF, NTOK], F16)
        hsq = qpool.tile([P, KF, NTOK], F16)

        # ---- DMA schedule (consumption order; 3 parallel queues) --------
        # Startup-critical bytes: all of x (re-read per fk from SBUF, so the
        # full 2.4MB gates fk0's last dk chunks) + fk0 weights.  Spread x
        # over all three queues; w2 rides sync/gpsimd AFTER w1/w3 (never on
        # scalar: the ACT queue is FIFO and must stay free for phase-A
        # sigmoids, which release PSUM).
        nc.scalar.dma_start(xsb[:, 0:2, :], xT_d[:, 0:2, :])
        nc.sync.dma_start(w1sb[:, 0, 0:8, :], w1_d[:, 0, 0:8, :])
        nc.gpsimd.dma_start(w3sb[:, 0, 0:8, :], w3_d[:, 0, 0:8, :])
        nc.scalar.dma_start(xsb[:, 2:4, :], xT_d[:, 2:4, :])
        nc.sync.dma_start(w1sb[:, 0, 8:KD, :], w1_d[:, 0, 8:KD, :])
        nc.gpsimd.dma_start(w3sb[:, 0, 8:KD, :], w3_d[:, 0, 8:KD, :])
        nc.scalar.dma_start(xsb[:, 4:6, :], xT_d[:, 4:6, :])
        nc.sync.dma_start(xsb[:, 6:10, :], xT_d[:, 6:10, :])
        nc.gpsimd.dma_start(xsb[:, 10:13, :], xT_d[:, 10:13, :])
        nc.gpsimd.dma_start(xsb[:, 13:KD, :], xT_d[:, 13:KD, :])
        for fk in range(1, KF):
            nc.sync.dma_start(w1sb[:, fk], w1_d[:, fk])
            nc.gpsimd.dma_start(w3sb[:, fk], w3_d[:, fk])
        for dk in range(0, KD, 4):
            q = nc.sync if (dk // 4) % 2 == 0 else nc.gpsimd
            q.dma_start(w2sb[:, dk : dk + 4], w2_d[:, dk : dk + 4])

        # HAM warm-up: keep the PE busy through the prologue DMA wait so
        # the clock gate is at 8/8 when the first real matmul's data lands.
        ps_w = psp.tile([P, 512], F32, tag="pa0", name="warm_ps")
        for _ in range(8):
            nc.tensor.matmul(ps_w[:], warm[:, 0:P], warm[:], start=True, stop=True)

        # ================= phase A: h = silu(x@w1^T) * (x@w3^T) ===========
        for fk in range(KF):
            p1 = [psp.tile([P, 512], F32, tag=f"pa{i}", name=f"p1_{fk}_{i}") for i in range(NCH)]
            p3 = [psp.tile([P, 512], F32, tag=f"pc{i}", name=f"p3_{fk}_{i}") for i in range(NCH)]
            for dk in range(KD):
                wc1 = w1sb[:, fk, dk, :]
                wc3 = w3sb[:, fk, dk, :]
                st = dk == 0
                sp = dk == KD - 1
                for i, (a, b) in enumerate(CH):
                    nc.tensor.matmul(
                        p1[i][:, 0 : b - a], wc1, xsb[:, dk, a:b], start=st, stop=sp
                    )
                for i, (a, b) in enumerate(CH):
                    nc.tensor.matmul(
                        p3[i][:, 0 : b - a], wc3, xsb[:, dk, a:b], start=st, stop=sp
                    )
            for i, (a, b) in enumerate(CH):
                w = b - a
                s = spool.tile([P, 512], F32, tag=f"sig{i}", name=f"sig_{fk}_{i}")
                nc.scalar.activation(s[:, 0:w], p1[i][:, 0:w], ACTF.Sigmoid)
                hs = h[:, fk, a:b]
                nc.vector.tensor_mul(hs, s[:, 0:w], p1[i][:, 0:w])
                nc.vector.tensor_mul(hs, hs, p3[i][:, 0:w])
                nc.vector.tensor_mul(hsq[:, fk, a:b], hs, hs)

        # ================= ssq[t] = sum_f h^2 (ones-column matmuls) =======
        sacc = [psp.tile([P, 512], F32, tag=f"pc{i}", name=f"sacc{i}") for i in range(NCH)]
        for fk in range(KF):
            for i, (a, b) in enumerate(CH):
                nc.tensor.matmul(
                    sacc[i][0:1, 0 : b - a],
                    ones[:],
                    hsq[:, fk, a:b],
                    start=(fk == 0),
                    stop=(fk == KF - 1),
                )
        for i, (a, b) in enumerate(CH):
            nc.vector.tensor_copy(ssq_sb[:, a:b], sacc[i][0:1, 0 : b - a])
        nc.scalar.dma_start(ssq_d[:, :], ssq_sb[:])

        # ================= phase C: out[d,t] = sum_fk w2c.T @ h ===========
        # output batches: big batches overlap phase C; tiny last batch so the
        # kernel end is not gated on a large transfer + completion receipt.
        OBATCH = [(0, 5), (5, 10), (10, 13), (13, 15), (15, 16)]
        obt = {
            g0: opool.tile([P, g1 - g0, NTOK], F16, tag=f"ob{g0}", name=f"ob{g0}")
            for g0, g1 in OBATCH
        }
        for gi, (g0, g1) in enumerate(OBATCH):
            ob = obt[g0]
            for dk in range(g0, g1):
                po = [psp.tile([P, 512], F32, tag=f"pa{i}", name=f"po_{dk}_{i}") for i in range(NCH)]
                for fk in range(KF):
                    wc2 = w2sb[:, dk, fk, :]
                    st = fk == 0
                    sp = fk == KF - 1
                    for i, (a, b) in enumerate(CH):
                        nc.tensor.matmul(
                            po[i][:, 0 : b - a], wc2, h[:, fk, a:b], start=st, stop=sp
                        )
                for i, (a, b) in enumerate(CH):
                    nc.vector.tensor_copy(ob[:, dk - g0, a:b], po[i][:, 0 : b - a])
            oq = nc.scalar if gi % 2 == 0 else nc.sync
            oq.dma_start(out_d[:, g0:g1, :], ob[:])

    nc.compile()
    return nc


def _get_program(NTOK: int):
    if NTOK not in _PROGRAM_CACHE:
        _PROGRAM_CACHE[NTOK] = _build_program(NTOK)
    return _PROGRAM_CACHE[NTOK]


def kernel(x, w1, w2, w3, mid_w, num_tokens_per_expert):
    global LAST_RESULTS
    x = np.ascontiguousarray(np.asarray(x, dtype=np.float32))
    w1 = np.asarray(w1, dtype=np.float32)
    w2 = np.asarray(w2, dtype=np.float32)
    w3 = np.asarray(w3, dtype=np.float32)
    mid_w = np.asarray(mid_w, dtype=np.float32)
    counts = np.asarray(num_tokens_per_expert).astype(np.int64)

    T_, D_ = x.shape
    E_, F_, _ = w1.shape
    Ccap = (T_ // E_) * 3 // 2  # reference static capacity (768)
    ends = np.cumsum(counts)
    starts = ends - counts
    eff = np.minimum(np.maximum(counts, 0), Ccap)  # rows actually computed

    NTOK = int(max(64, -(-int(eff.max()) // 64) * 64))  # pad to 64 tokens
    nc = _get_program(NTOK)

    in_maps = []
    for e in range(E_):
        cnt = int(eff[e])
        s = int(starts[e])
        xg = np.zeros((NTOK, D_), np.float32)
        if cnt > 0:
            rows = np.clip(s + np.arange(cnt), 0, T_ - 1)
            xg[:cnt] = x[rows]
        # xT: [P(p), KD, NTOK] with [p, dk, t] = x[t, dk*128+p]
        xT = xg.T.astype(np.float16).reshape(KD, P, NTOK).transpose(1, 0, 2)
        # w1t/w3t: [p, fk, dk, q] = w[fk*128+q, dk*128+p]   (w is [F, D])
        w1t = w1[e].astype(np.float16).reshape(KF, P, KD, P).transpose(3, 0, 2, 1)
        w3t = w3[e].astype(np.float16).reshape(KF, P, KD, P).transpose(3, 0, 2, 1)
        # w2t: [p, dk, fk, q] = w2m[dk*128+q, fk*128+p]     (w2m is [D, F])
        w2m = w2[e] * mid_w[None, :]
        w2t = w2m.astype(np.float16).reshape(KD, P, KF, P).transpose(3, 0, 2, 1)
        in_maps.append(
            {
                "xT": np.ascontiguousarray(xT),
                "w1t": np.ascontiguousarray(w1t),
                "w3t": np.ascontiguousarray(w3t),
                "w2t": np.ascontiguousarray(w2t),
            }
        )

    LAST_RESULTS = _run(nc, in_maps)

    outs = []
    for e in range(E_):
        o = (
            np.asarray(LAST_RESULTS[e]["out"], np.float32)
            .transpose(1, 0, 2)
            .reshape(D_, NTOK)
            .T
        )
        ssq = np.asarray(LAST_RESULTS[e]["ssq"], np.float32).reshape(NTOK)
        rstd = 1.0 / np.sqrt(ssq / F_ + EPS)
        outs.append(o * rstd[:, None])  # [NTOK, D]
    stacked = np.stack(outs, axis=0)  # [E, NTOK, D]

    # scatter back to flat token order, mirroring the reference's clamping
    tok = np.arange(T_)
    eid = np.clip(np.searchsorted(ends, tok, side="right"), 0, E_ - 1)
    pos = tok - starts[eid]
    idx = np.minimum(pos, Ccap - 1)
    valid = (idx >= 0) & (idx < eff[eid])
    idx_safe = np.clip(idx, 0, NTOK - 1)
    result = stacked[eid, idx_safe].astype(np.float32)
    result[~valid] = 0.0
    return result


# revision 8
# speedup vs baseline: 1.0929x; 1.0039x over previous
"""Trainium2 Bass kernel: grouped-experts SwiGLU MLP with mid-RMSNorm.

Expert-parallel across 8 NeuronCores: core e computes expert e's token
block (tokens are pre-sorted by expert).

v2: weight-stationary ("flipped") matmul orientation.  The moving
operand is the token axis, so PE cost scales with the actual padded
token count (576, vs 640 tile-padded in v1) and the SwiGLU hidden state
is produced directly in [f, tok] layout -- exactly what the down-proj
matmul consumes -- eliminating all PE transposes.  The mid-RMSNorm
row scale commutes with the (linear) down projection, so the device
only produces ssq[tok] = sum_f h^2 (via a ones-column matmul) and the
host applies rstd; mid_w is folded into w2.

Per-core math (fp16 in / fp32 accumulate), NTOK = 576 padded tokens:
    h1[f,t] = sum_dk w1c[dk].T @ x[dk]     (16 accum MMs per f-chunk)
    h  = silu(h1) * h3                     # [128, KF, NTOK] in SBUF
    ssq[t] = ones.T @ (h*h)                # [1, NTOK] accum over KF
    out[d,t] = sum_fk w2c[fk].T @ h[fk]    # [KD, 128, NTOK]
Host: out_rows *= rsqrt(ssq/F + eps); scatter to flat token order.

PSUM: one pool, 4 tags x full 2KB bank x 2 bufs = 8 banks exactly; no
two accumulation tiles share a bank, so PE-write never collides with
ScalarE/VectorE reads of a neighbouring tile.

DMA: w1 on sync, w3 on gpsimd, x + w2 on scalar (w2 is only needed for
phase C at ~2/3 of the kernel), outputs alternate sync/gpsimd.  First
slabs of x and w1/w3 are split small so the first matmuls gate early;
~12 warm-up matmuls cover the initial DMA wait and HAM ramp.
"""

import sys

sys.path.insert(0, "/opt/trn_rl_repo")

import os

import numpy as np
from contextlib import ExitStack

import concourse.bass as bass
import concourse.tile as tile
from concourse import bacc, mybir

P = 128
D = 2048
F = 1024
E = 8
KD = D // P  # 16 contraction chunks for mm1/mm3
KF = F // P  # 8 f chunks (contraction chunks for mm2)
EPS = 1e-6
F32 = mybir.dt.float32
F16 = mybir.dt.float16
ACTF = mybir.ActivationFunctionType

_PROGRAM_CACHE: dict[int, object] = {}
LAST_RESULTS = None  # test harness reads per-core outputs from here


def _run(nc, in_maps):
    """Execute the compiled program on the 8 axon-tunneled cores.

    If KERNEL_NTFF_DIR is set, wrap the execute in the axon NTFF profile
    hook so device profiles land there (test harness use only).
    """
    from concourse import bass2jax

    ntff_dir = os.environ.get("KERNEL_NTFF_DIR")
    if ntff_dir:
        if "/root/.axon_site" not in sys.path:
            sys.path.insert(0, "/root/.axon_site")
        from trn_agent_boot.trn_boot import _ntff_profile_via_ctypes

        hook = _ntff_profile_via_ctypes("/opt/axon/libaxon_pjrt.so")
        ids = [
            int(x) for x in os.environ.get("KERNEL_NTFF_CORES", "0").split(",")
        ]
        if hook is not None:
            with hook(ntff_dir, ids):
                return bass2jax.run_bass_via_pjrt(nc, in_maps, n_cores=len(in_maps))
    return bass2jax.run_bass_via_pjrt(nc, in_maps, n_cores=len(in_maps))


def _build_program(NTOK: int):
    """Build + compile the single-core SPMD program for NTOK padded tokens."""
    # token chunks, each <= 512 (one fp32 PSUM bank of moving dim)
    CH = [(0, min(512, NTOK))]
    if NTOK > 512:
        CH.append((512, NTOK))
    NCH = len(CH)

    nc = bacc.Bacc(
        "TRN2",
        target_bir_lowering=False,
        debug=False,
        enable_asserts=False,
        num_devices=E,
    )
    xT_d = nc.dram_tensor("xT", [P, KD, NTOK], F16, kind="ExternalInput").ap()
    w1_d = nc.dram_tensor("w1t", [P, KF, KD, P], F16, kind="ExternalInput").ap()
    w3_d = nc.dram_tensor("w3t", [P, KF, KD, P], F16, kind="ExternalInput").ap()
    w2_d = nc.dram_tensor("w2t", [P, KD, KF, P], F16, kind="ExternalInput").ap()
    out_d = nc.dram_tensor("out", [P, KD, NTOK], F16, kind="ExternalOutput").ap()
    ssq_d = nc.dram_tensor("ssq", [1, NTOK], F32, kind="ExternalOutput").ap()

    with tile.TileContext(nc) as tc, ExitStack() as ctx:
        singles = ctx.enter_context(tc.tile_pool(name="singles", bufs=1))
        xpool = ctx.enter_context(tc.tile_pool(name="x", bufs=1))
        w1pool = ctx.enter_context(tc.tile_pool(name="w1", bufs=1))
        w3pool = ctx.enter_context(tc.tile_pool(name="w3", bufs=1))
        w2pool = ctx.enter_context(tc.tile_pool(name="w2", bufs=1))
        hpool = ctx.enter_context(tc.tile_pool(name="h", bufs=1))
        qpool = ctx.enter_context(tc.tile_pool(name="hsq", bufs=1))
        spool = ctx.enter_context(tc.tile_pool(name="scr", bufs=2))
        opool = ctx.enter_context(tc.tile_pool(name="o", bufs=1))
        psp = ctx.enter_context(tc.tile_pool(name="ps", bufs=2, space="PSUM"))

        warm = singles.tile([P, 512], F16, name="warm")
        nc.gpsimd.memset(warm[:], 0.5)
        ones = singles.tile([P, 1], F16, name="ones")
        nc.gpsimd.memset(ones[:], 1.0)
        ssq_sb = singles.tile([1, NTOK], F32, name="ssq_sb")

        xsb = xpool.tile([P, KD, NTOK], F16)
        w1sb = w1pool.tile([P, KF, KD, P], F16)
        w3sb = w3pool.tile([P, KF, KD, P], F16)
        w2sb = w2pool.tile([P, KD, KF, P], F16)
        h = hpool.tile([P, KF, NTOK], F16)
        hsq = qpool.tile([P, KF, NTOK], F16)

        # ---- DMA schedule (consumption order; 3 parallel queues) --------
        # Startup-critical bytes: all of x (re-read per fk from SBUF, so the
        # full 2.4MB gates fk0's last dk chunks) + fk0 weights.  Spread x
        # over all three queues; w2 rides sync/gpsimd AFTER w1/w3 (never on
        # scalar: the ACT queue is FIFO and must stay free for phase-A
        # sigmoids, which release PSUM).
        nc.scalar.dma_start(xsb[:, 0:2, :], xT_d[:, 0:2, :])
        nc.sync.dma_start(w1sb[:, 0, 0:4, :], w1_d[:, 0, 0:4, :])
        nc.gpsimd.dma_start(w3sb[:, 0, 0:4, :], w3_d[:, 0, 0:4, :])
        nc.scalar.dma_start(xsb[:, 2:4, :], xT_d[:, 2:4, :])
        nc.sync.dma_start(w1sb[:, 0, 4:KD, :], w1_d[:, 0, 4:KD, :])
        nc.gpsimd.dma_start(w3sb[:, 0, 4:KD, :], w3_d[:, 0, 4:KD, :])
        nc.scalar.dma_start(xsb[:, 4:6, :], xT_d[:, 4:6, :])
        nc.sync.dma_start(xsb[:, 6:10, :], xT_d[:, 6:10, :])
        nc.gpsimd.dma_start(xsb[:, 10:13, :], xT_d[:, 10:13, :])
        nc.gpsimd.dma_start(xsb[:, 13:KD, :], xT_d[:, 13:KD, :])
        for fk in range(1, KF):
            nc.sync.dma_start(w1sb[:, fk], w1_d[:, fk])
            nc.gpsimd.dma_start(w3sb[:, fk], w3_d[:, fk])
        for dk in range(0, KD, 4):
            q = nc.sync if (dk // 4) % 2 == 0 else nc.gpsimd
            q.dma_start(w2sb[:, dk : dk + 4], w2_d[:, dk : dk + 4])

        # HAM warm-up: keep the PE busy through the prologue DMA wait so
        # the clock gate is at 8/8 when the first real matmul's data lands.
        ps_w = psp.tile([P, 512], F32, tag="pa0", name="warm_ps")
        for _ in range(2):
            nc.tensor.matmul(ps_w[:], warm[:, 0:P], warm[:], start=True, stop=True)

        # ================= phase A: h = silu(x@w1^T) * (x@w3^T) ===========
        for fk in range(KF):
            p1 = [psp.tile([P, 512], F32, tag=f"pa{i}", name=f"p1_{fk}_{i}") for i in range(NCH)]
            p3 = [psp.tile([P, 512], F32, tag=f"pc{i}", name=f"p3_{fk}_{i}") for i in range(NCH)]
            for dk in range(KD):
                wc1 = w1sb[:, fk, dk, :]
                wc3 = w3sb[:, fk, dk, :]
                st = dk == 0
                sp = dk == KD - 1
                for i, (a, b) in enumerate(CH):
                    nc.tensor.matmul(
                        p1[i][:, 0 : b - a], wc1, xsb[:, dk, a:b], start=st, stop=sp
                    )
                for i, (a, b) in enumerate(CH):
                    nc.tensor.matmul(
                        p3[i][:, 0 : b - a], wc3, xsb[:, dk, a:b], start=st, stop=sp
                    )
            for i, (a, b) in enumerate(CH):
                w = b - a
                s = spool.tile([P, 512], F32, tag=f"sig{i}", name=f"sig_{fk}_{i}")
                nc.scalar.activation(s[:, 0:w], p1[i][:, 0:w], ACTF.Sigmoid)
                hs = h[:, fk, a:b]
                nc.vector.tensor_mul(hs, s[:, 0:w], p1[i][:, 0:w])
                nc.vector.tensor_mul(hs, hs, p3[i][:, 0:w])
                nc.vector.tensor_mul(hsq[:, fk, a:b], hs, hs)

        # ================= ssq[t] = sum_f h^2 (ones-column matmuls) =======
        sacc = [psp.tile([P, 512], F32, tag=f"pc{i}", name=f"sacc{i}") for i in range(NCH)]
        for fk in range(KF):
            for i, (a, b) in enumerate(CH):
                nc.tensor.matmul(
                    sacc[i][0:1, 0 : b - a],
                    ones[:],
                    hsq[:, fk, a:b],
                    start=(fk == 0),
                    stop=(fk == KF - 1),
                )
        for i, (a, b) in enumerate(CH):
            nc.vector.tensor_copy(ssq_sb[:, a:b], sacc[i][0:1, 0 : b - a])
        nc.scalar.dma_start(ssq_d[:, :], ssq_sb[:])

        # ================= phase C: out[d,t] = sum_fk w2c.T @ h ===========
        # output batches: big batches overlap phase C; tiny last batch so the
        # kernel end is not gated on a large transfer + completion receipt.
        OBATCH = [(0, 5), (5, 10), (10, 13), (13, 15), (15, 16)]
        obt = {
            g0: opool.tile([P, g1 - g0, NTOK], F16, tag=f"ob{g0}", name=f"ob{g0}")
            for g0, g1 in OBATCH
        }
        for gi, (g0, g1) in enumerate(OBATCH):
            ob = obt[g0]
            for dk in range(g0, g1):
                po = [psp.tile([P, 512], F32, tag=f"pa{i}", name=f"po_{dk}_{i}") for i in range(NCH)]
                for fk in range(KF):
                    wc2 = w2sb[:, dk, fk, :]
                    st = fk == 0
                    sp = fk == KF - 1
                    for i, (a, b) in enumerate(CH):
                        nc.tensor.matmul(
                            po[i][:, 0 : b - a], wc2, h[:, fk, a:b], start=st, stop=sp
                        )
                for i, (a, b) in enumerate(CH):
                    nc.vector.tensor_copy(ob[:, dk - g0, a:b], po[i][:, 0 : b - a])
            oq = nc.scalar if gi % 2 == 0 else nc.sync
            oq.dma_start(out_d[:, g0:g1, :], ob[:])

    nc.compile()
    return nc


def _get_program(NTOK: int):
    if NTOK not in _PROGRAM_CACHE:
        _PROGRAM_CACHE[NTOK] = _build_program(NTOK)
    return _PROGRAM_CACHE[NTOK]


def kernel(x, w1, w2, w3, mid_w, num_tokens_per_expert):
    global LAST_RESULTS
    x = np.ascontiguousarray(np.asarray(x, dtype=np.float32))
    w1 = np.asarray(w1, dtype=np.float32)
    w2 = np.asarray(w2, dtype=np.float32)
    w3 = np.asarray(w3, dtype=np.float32)
    mid_w = np.asarray(mid_w, dtype=np.float32)
    counts = np.asarray(num_tokens_per_expert).astype(np.int64)

    T_, D_ = x.shape
    E_, F_, _ = w1.shape
    Ccap = (T_ // E_) * 3 // 2  # reference static capacity (768)
    ends = np.cumsum(counts)
    starts = ends - counts
    eff = np.minimum(np.maximum(counts, 0), Ccap)  # rows actually computed

    NTOK = int(max(64, -(-int(eff.max()) // 64) * 64))  # pad to 64 tokens
    nc = _get_program(NTOK)

    in_maps = []
    for e in range(E_):
        cnt = int(eff[e])
        s = int(starts[e])
        xg = np.zeros((NTOK, D_), np.float32)
        if cnt > 0:
            rows = np.clip(s + np.arange(cnt), 0, T_ - 1)
            xg[:cnt] = x[rows]
        # xT: [P(p), KD, NTOK] with [p, dk, t] = x[t, dk*128+p]
        xT = xg.T.astype(np.float16).reshape(KD, P, NTOK).transpose(1, 0, 2)
        # w1t/w3t: [p, fk, dk, q] = w[fk*128+q, dk*128+p]   (w is [F, D])
        w1t = w1[e].astype(np.float16).reshape(KF, P, KD, P).transpose(3, 0, 2, 1)
        w3t = w3[e].astype(np.float16).reshape(KF, P, KD, P).transpose(3, 0, 2, 1)
        # w2t: [p, dk, fk, q] = w2m[dk*128+q, fk*128+p]     (w2m is [D, F])
        w2m = w2[e] * mid_w[None, :]
        w2t = w2m.astype(np.float16).reshape(KD, P, KF, P).transpose(3, 0, 2, 1)
        in_maps.append(
            {
                "xT": np.ascontiguousarray(xT),
                "w1t": np.ascontiguousarray(w1t),
                "w3t": np.ascontiguousarray(w3t),
                "w2t": np.ascontiguousarray(w2t),
            }
        )

    LAST_RESULTS = _run(nc, in_maps)

    outs = []
    for e in range(E_):
        o = (
            np.asarray(LAST_RESULTS[e]["out"], np.float32)
            .transpose(1, 0, 2)
            .reshape(D_, NTOK)
            .T
        )
        ssq = np.asarray(LAST_RESULTS[e]["ssq"], np.float32).reshape(NTOK)
        rstd = 1.0 / np.sqrt(ssq / F_ + EPS)
        outs.append(o * rstd[:, None])  # [NTOK, D]
    stacked = np.stack(outs, axis=0)  # [E, NTOK, D]

    # scatter back to flat token order, mirroring the reference's clamping
    tok = np.arange(T_)
    eid = np.clip(np.searchsorted(ends, tok, side="right"), 0, E_ - 1)
    pos = tok - starts[eid]
    idx = np.minimum(pos, Ccap - 1)
    valid = (idx >= 0) & (idx < eff[eid])
    idx_safe = np.clip(idx, 0, NTOK - 1)
    result = stacked[eid, idx_safe].astype(np.float32)
    result[~valid] = 0.0
    return result


# revision 9
# speedup vs baseline: 1.0943x; 1.0012x over previous
"""Trainium2 Bass kernel: grouped-experts SwiGLU MLP with mid-RMSNorm.

Expert-parallel across 8 NeuronCores: core e computes expert e's token
block (tokens are pre-sorted by expert).

v2: weight-stationary ("flipped") matmul orientation.  The moving
operand is the token axis, so PE cost scales with the actual padded
token count (576, vs 640 tile-padded in v1) and the SwiGLU hidden state
is produced directly in [f, tok] layout -- exactly what the down-proj
matmul consumes -- eliminating all PE transposes.  The mid-RMSNorm
row scale commutes with the (linear) down projection, so the device
only produces ssq[tok] = sum_f h^2 (via a ones-column matmul) and the
host applies rstd; mid_w is folded into w2.

Per-core math (fp16 in / fp32 accumulate), NTOK = 576 padded tokens:
    h1[f,t] = sum_dk w1c[dk].T @ x[dk]     (16 accum MMs per f-chunk)
    h  = silu(h1) * h3                     # [128, KF, NTOK] in SBUF
    ssq[t] = ones.T @ (h*h)                # [1, NTOK] accum over KF
    out[d,t] = sum_fk w2c[fk].T @ h[fk]    # [KD, 128, NTOK]
Host: out_rows *= rsqrt(ssq/F + eps); scatter to flat token order.

PSUM: one pool, 4 tags x full 2KB bank x 2 bufs = 8 banks exactly; no
two accumulation tiles share a bank, so PE-write never collides with
ScalarE/VectorE reads of a neighbouring tile.

DMA: w1 on sync, w3 on gpsimd, x + w2 on scalar (w2 is only needed for
phase C at ~2/3 of the kernel), outputs alternate sync/gpsimd.  First
slabs of x and w1/w3 are split small so the first matmuls gate early;
~12 warm-up matmuls cover the initial DMA wait and HAM ramp.
"""

import sys

sys.path.insert(0, "/opt/trn_rl_repo")

import os

import numpy as np
from contextlib import ExitStack

import concourse.bass as bass
import concourse.tile as tile
from concourse import bacc, mybir

P = 128
D = 2048
F = 1024
E = 8
KD = D // P  # 16 contraction chunks for mm1/mm3
KF = F // P  # 8 f chunks (contraction chunks for mm2)
EPS = 1e-6
F32 = mybir.dt.float32
F16 = mybir.dt.float16
ACTF = mybir.ActivationFunctionType

_PROGRAM_CACHE: dict[int, object] = {}
LAST_RESULTS = None  # test harness reads per-core outputs from here


def _run(nc, in_maps):
    """Execute the compiled program on the 8 axon-tunneled cores.

    If KERNEL_NTFF_DIR is set, wrap the execute in the axon NTFF profile
    hook so device profiles land there (test harness use only).
    """
    from concourse import bass2jax

    ntff_dir = os.environ.get("KERNEL_NTFF_DIR")
    if ntff_dir:
        if "/root/.axon_site" not in sys.path:
            sys.path.insert(0, "/root/.axon_site")
        from trn_agent_boot.trn_boot import _ntff_profile_via_ctypes

        hook = _ntff_profile_via_ctypes("/opt/axon/libaxon_pjrt.so")
        ids = [
            int(x) for x in os.environ.get("KERNEL_NTFF_CORES", "0").split(",")
        ]
        if hook is not None:
            with hook(ntff_dir, ids):
                return bass2jax.run_bass_via_pjrt(nc, in_maps, n_cores=len(in_maps))
    return bass2jax.run_bass_via_pjrt(nc, in_maps, n_cores=len(in_maps))


def _build_program(NTOK: int):
    """Build + compile the single-core SPMD program for NTOK padded tokens."""
    # token chunks, each <= 512 (one fp32 PSUM bank of moving dim)
    CH = [(0, min(512, NTOK))]
    if NTOK > 512:
        CH.append((512, NTOK))
    NCH = len(CH)

    nc = bacc.Bacc(
        "TRN2",
        target_bir_lowering=False,
        debug=False,
        enable_asserts=False,
        num_devices=E,
    )
    xT_d = nc.dram_tensor("xT", [P, KD, NTOK], F16, kind="ExternalInput").ap()
    w1_d = nc.dram_tensor("w1t", [P, KF, KD, P], F16, kind="ExternalInput").ap()
    w3_d = nc.dram_tensor("w3t", [P, KF, KD, P], F16, kind="ExternalInput").ap()
    w2_d = nc.dram_tensor("w2t", [P, KD, KF, P], F16, kind="ExternalInput").ap()
    out_d = nc.dram_tensor("out", [P, KD, NTOK], F16, kind="ExternalOutput").ap()
    ssq_d = nc.dram_tensor("ssq", [1, NTOK], F32, kind="ExternalOutput").ap()

    with tile.TileContext(nc) as tc, ExitStack() as ctx:
        singles = ctx.enter_context(tc.tile_pool(name="singles", bufs=1))
        xpool = ctx.enter_context(tc.tile_pool(name="x", bufs=1))
        w1pool = ctx.enter_context(tc.tile_pool(name="w1", bufs=1))
        w3pool = ctx.enter_context(tc.tile_pool(name="w3", bufs=1))
        w2pool = ctx.enter_context(tc.tile_pool(name="w2", bufs=1))
        hpool = ctx.enter_context(tc.tile_pool(name="h", bufs=1))
        qpool = ctx.enter_context(tc.tile_pool(name="hsq", bufs=1))
        spool = ctx.enter_context(tc.tile_pool(name="scr", bufs=2))
        opool = ctx.enter_context(tc.tile_pool(name="o", bufs=1))
        psp = ctx.enter_context(tc.tile_pool(name="ps", bufs=2, space="PSUM"))

        warm = singles.tile([P, 512], F16, name="warm")
        nc.gpsimd.memset(warm[:], 0.5)
        ones = singles.tile([P, 1], F16, name="ones")
        nc.gpsimd.memset(ones[:], 1.0)
        ssq_sb = singles.tile([1, NTOK], F32, name="ssq_sb")

        xsb = xpool.tile([P, KD, NTOK], F16)
        w1sb = w1pool.tile([P, KF, KD, P], F16)
        w3sb = w3pool.tile([P, KF, KD, P], F16)
        w2sb = w2pool.tile([P, KD, KF, P], F16)
        h = hpool.tile([P, KF, NTOK], F16)
        hsq = qpool.tile([P, KF, NTOK], F16)

        # ---- DMA schedule (consumption order; 3 parallel queues) --------
        # Startup-critical bytes: all of x (re-read per fk from SBUF, so the
        # full 2.4MB gates fk0's last dk chunks) + fk0 weights.  Spread x
        # over all three queues; w2 rides sync/gpsimd AFTER w1/w3 (never on
        # scalar: the ACT queue is FIFO and must stay free for phase-A
        # sigmoids, which release PSUM).
        nc.scalar.dma_start(xsb[:, 0:2, :], xT_d[:, 0:2, :])
        nc.sync.dma_start(w1sb[:, 0, 0:4, :], w1_d[:, 0, 0:4, :])
        nc.gpsimd.dma_start(w3sb[:, 0, 0:4, :], w3_d[:, 0, 0:4, :])
        nc.scalar.dma_start(xsb[:, 2:4, :], xT_d[:, 2:4, :])
        nc.sync.dma_start(w1sb[:, 0, 4:KD, :], w1_d[:, 0, 4:KD, :])
        nc.gpsimd.dma_start(w3sb[:, 0, 4:KD, :], w3_d[:, 0, 4:KD, :])
        nc.scalar.dma_start(xsb[:, 4:6, :], xT_d[:, 4:6, :])
        nc.sync.dma_start(xsb[:, 6:10, :], xT_d[:, 6:10, :])
        nc.gpsimd.dma_start(xsb[:, 10:13, :], xT_d[:, 10:13, :])
        nc.gpsimd.dma_start(xsb[:, 13:KD, :], xT_d[:, 13:KD, :])
        for fk in range(1, KF):
            nc.sync.dma_start(w1sb[:, fk], w1_d[:, fk])
            nc.gpsimd.dma_start(w3sb[:, fk], w3_d[:, fk])
        for dk in range(0, KD, 4):
            q = nc.sync if (dk // 4) % 2 == 0 else nc.gpsimd
            q.dma_start(w2sb[:, dk : dk + 4], w2_d[:, dk : dk + 4])

        # HAM warm-up: keep the PE busy through the prologue DMA wait so
        # the clock gate is at 8/8 when the first real matmul's data lands.
        PSW = 1024 if NTOK > 512 else 512  # psum tile: 2 banks if tail chunk
        ps_w = psp.tile([P, PSW], F32, tag="pa", name="warm_ps")
        for _ in range(5):
            nc.tensor.matmul(ps_w[:, 0:512], warm[:, 0:P], warm[:], start=True, stop=True)

        # ================= phase A: h = silu(x@w1^T) * (x@w3^T) ===========
        # One [P, PSW] psum tile per h1/h3: the 512-wide chunk lands in the
        # first bank, the tail in the second, so the epilogue runs as single
        # wide ops over [0:NTOK] (fewer instructions + semaphore edges).
        for fk in range(KF):
            p1 = psp.tile([P, PSW], F32, tag="pa", name=f"p1_{fk}")
            p3 = psp.tile([P, PSW], F32, tag="pc", name=f"p3_{fk}")
            for dk in range(KD):
                wc1 = w1sb[:, fk, dk, :]
                wc3 = w3sb[:, fk, dk, :]
                st = dk == 0
                sp = dk == KD - 1
                for a, b in CH:
                    nc.tensor.matmul(p1[:, a:b], wc1, xsb[:, dk, a:b], start=st, stop=sp)
                for a, b in CH:
                    nc.tensor.matmul(p3[:, a:b], wc3, xsb[:, dk, a:b], start=st, stop=sp)
            s = spool.tile([P, NTOK], F32, tag="sig", name=f"sig_{fk}")
            nc.scalar.activation(s[:], p1[:, 0:NTOK], ACTF.Sigmoid)
            hs = h[:, fk, :]
            nc.vector.tensor_mul(hs, s[:], p1[:, 0:NTOK])
            nc.vector.tensor_mul(hs, hs, p3[:, 0:NTOK])
            nc.vector.tensor_mul(hsq[:, fk, :], hs, hs)

        # ================= ssq[t] = sum_f h^2 (ones-column matmuls) =======
        sacc = psp.tile([P, PSW], F32, tag="pc", name="sacc")
        for fk in range(KF):
            for a, b in CH:
                nc.tensor.matmul(
                    sacc[0:1, a:b],
                    ones[:],
                    hsq[:, fk, a:b],
                    start=(fk == 0),
                    stop=(fk == KF - 1),
                )
        nc.vector.tensor_copy(ssq_sb[:], sacc[0:1, 0:NTOK])
        nc.scalar.dma_start(ssq_d[:, :], ssq_sb[:])

        # ================= phase C: out[d,t] = sum_fk w2c.T @ h ===========
        # output batches: big batches overlap phase C; tiny last batch so the
        # kernel end is not gated on a large transfer + completion receipt.
        OBATCH = [(0, 5), (5, 10), (10, 13), (13, 15), (15, 16)]
        obt = {
            g0: opool.tile([P, g1 - g0, NTOK], F16, tag=f"ob{g0}", name=f"ob{g0}")
            for g0, g1 in OBATCH
        }
        for gi, (g0, g1) in enumerate(OBATCH):
            ob = obt[g0]
            for dk in range(g0, g1):
                po = psp.tile([P, PSW], F32, tag="pa" if dk % 2 == 0 else "pc", name=f"po_{dk}")
                for fk in range(KF):
                    wc2 = w2sb[:, dk, fk, :]
                    st = fk == 0
                    sp = fk == KF - 1
                    for a, b in CH:
                        nc.tensor.matmul(po[:, a:b], wc2, h[:, fk, a:b], start=st, stop=sp)
                nc.vector.tensor_copy(ob[:, dk - g0, :], po[:, 0:NTOK])
            oq = nc.scalar if gi % 2 == 0 else nc.sync
            oq.dma_start(out_d[:, g0:g1, :], ob[:])

    nc.compile()
    return nc


def _get_program(NTOK: int):
    if NTOK not in _PROGRAM_CACHE:
        _PROGRAM_CACHE[NTOK] = _build_program(NTOK)
    return _PROGRAM_CACHE[NTOK]


def kernel(x, w1, w2, w3, mid_w, num_tokens_per_expert):
    global LAST_RESULTS
    x = np.ascontiguousarray(np.asarray(x, dtype=np.float32))
    w1 = np.asarray(w1, dtype=np.float32)
    w2 = np.asarray(w2, dtype=np.float32)
    w3 = np.asarray(w3, dtype=np.float32)
    mid_w = np.asarray(mid_w, dtype=np.float32)
    counts = np.asarray(num_tokens_per_expert).astype(np.int64)

    T_, D_ = x.shape
    E_, F_, _ = w1.shape
    Ccap = (T_ // E_) * 3 // 2  # reference static capacity (768)
    ends = np.cumsum(counts)
    starts = ends - counts
    eff = np.minimum(np.maximum(counts, 0), Ccap)  # rows actually computed

    NTOK = int(max(64, -(-int(eff.max()) // 64) * 64))  # pad to 64 tokens
    nc = _get_program(NTOK)

    in_maps = []
    for e in range(E_):
        cnt = int(eff[e])
        s = int(starts[e])
        xg = np.zeros((NTOK, D_), np.float32)
        if cnt > 0:
            rows = np.clip(s + np.arange(cnt), 0, T_ - 1)
            xg[:cnt] = x[rows]
        # xT: [P(p), KD, NTOK] with [p, dk, t] = x[t, dk*128+p]
        xT = xg.T.astype(np.float16).reshape(KD, P, NTOK).transpose(1, 0, 2)
        # w1t/w3t: [p, fk, dk, q] = w[fk*128+q, dk*128+p]   (w is [F, D])
        w1t = w1[e].astype(np.float16).reshape(KF, P, KD, P).transpose(3, 0, 2, 1)
        w3t = w3[e].astype(np.float16).reshape(KF, P, KD, P).transpose(3, 0, 2, 1)
        # w2t: [p, dk, fk, q] = w2m[dk*128+q, fk*128+p]     (w2m is [D, F])
        w2m = w2[e] * mid_w[None, :]
        w2t = w2m.astype(np.float16).reshape(KD, P, KF, P).transpose(3, 0, 2, 1)
        in_maps.append(
            {
                "xT": np.ascontiguousarray(xT),
                "w1t": np.ascontiguousarray(w1t),
                "w3t": np.ascontiguousarray(w3t),
                "w2t": np.ascontiguousarray(w2t),
            }
        )

    LAST_RESULTS = _run(nc, in_maps)

    outs = []
    for e in range(E_):
        o = (
            np.asarray(LAST_RESULTS[e]["out"], np.float32)
            .transpose(1, 0, 2)
            .reshape(D_, NTOK)
            .T
        )
        ssq = np.asarray(LAST_RESULTS[e]["ssq"], np.float32).reshape(NTOK)
        rstd = 1.0 / np.sqrt(ssq / F_ + EPS)
        outs.append(o * rstd[:, None])  # [NTOK, D]
    stacked = np.stack(outs, axis=0)  # [E, NTOK, D]

    # scatter back to flat token order, mirroring the reference's clamping
    tok = np.arange(T_)
    eid = np.clip(np.searchsorted(ends, tok, side="right"), 0, E_ - 1)
    pos = tok - starts[eid]
    idx = np.minimum(pos, Ccap - 1)
    valid = (idx >= 0) & (idx < eff[eid])
    idx_safe = np.clip(idx, 0, NTOK - 1)
    result = stacked[eid, idx_safe].astype(np.float32)
    result[~valid] = 0.0
    return result
